# revision 1
# baseline (speedup 1.0000x reference)
"""BitNet attention TRN2 kernel: 8-core SPMD (2 batch groups x 4 head groups).

Per core cid = 4*g + j (g = batch index, j = head-group index):
  - ternary-quantized QKV projections for heads [4j, 4j+4) of batch g
    (hi/lo-split bf16 matmuls for fp32-grade q/k; v in single-pass bf16).
    Weights arrive host-pre-transposed so ternarization needs no on-chip
    transpose; x hi/lo tiles are transposed via the DMA xbar (no PE).
  - attention: scores hi/lo bf16 into one 4-bank PSUM tile, single fused
    row-max + exp(+denominator), bf16 probs, DMA-xbar probs transpose,
  - partial attn-mean accumulated fp32, written bf16, per-qb
    ReduceScattered over the 4-core batch group,
  - output projection computed as a partial sum over the core's 512
    attended dims (wo input-sliced), per-qb ReduceScattered (no AllGather).
BitNet per-tensor scales: each core reduces |w| over a distinct 256-row slab
of each weight; one tiny 8-core AllReduce yields the full-tensor means.
"""

import os

import numpy as np

os.environ.setdefault("NEURON_RT_RESET_CORES", "1")

B, S, D, H = 2, 2048, 2048, 16
HD = D // H            # 128 head dim
HG = H // 4            # 4 heads per core
OS = HG * HD           # 512-wide slice per core
P = 128
NCORES = 8
NDT = D // P           # 16 contraction tiles
C_SCALE = np.float32(1.0 / np.sqrt(HD))
THRESH = np.float32(2.0 / 3.0)

_CACHE = {}


def _build(use_mask: bool, single: bool = False, phases: int = 3):
    import concourse.mybir as mybir
    import concourse.tile as tile
    from concourse import bacc

    F32 = mybir.dt.float32
    BF16 = mybir.dt.bfloat16
    I32 = mybir.dt.int32
    AX = mybir.AxisListType
    ALU = mybir.AluOpType
    ACTF = mybir.ActivationFunctionType

    nc = bacc.Bacc("TRN2", target_bir_lowering=False, debug=False,
                   num_devices=1 if single else NCORES)

    def cc(kind, op, groups, ins, outs):
        if not single:
            nc.gpsimd.collective_compute(kind, op, replica_groups=groups,
                                         ins=ins, outs=outs)
            return
        # timing-only single-core substitute: local DMA of this core's part
        src_ap, dst_ap = ins[0], outs[0]
        if kind == "AllGather":
            nc.gpsimd.dma_start(out=dst_ap[0:src_ap.shape[0]], in_=src_ap)
        elif kind == "ReduceScatter":
            nc.gpsimd.dma_start(out=dst_ap, in_=src_ap[0:dst_ap.shape[0]])
        else:
            nc.gpsimd.dma_start(out=dst_ap, in_=src_ap)

    # ---- I/O ----
    xq_d = nc.dram_tensor("xq", [S, D], F32, kind="ExternalInput")
    xk_d = nc.dram_tensor("xk", [S, D], F32, kind="ExternalInput")
    xv_d = nc.dram_tensor("xv", [S, D], F32, kind="ExternalInput")
    wslab_d = nc.dram_tensor("w_slab", [4, 256, D], F32, kind="ExternalInput")
    # host-pre-transposed weight slices: [D, OS] for q/k/v, [OS, D] for o
    w_in = {
        "q": nc.dram_tensor("wqT_s", [D, OS], F32, kind="ExternalInput"),
        "k": nc.dram_tensor("wkT_s", [D, OS], F32, kind="ExternalInput"),
        "v": nc.dram_tensor("wvT_s", [D, OS], F32, kind="ExternalInput"),
        "o": nc.dram_tensor("woT_s", [OS, D], F32, kind="ExternalInput"),
    }
    bq_d = nc.dram_tensor("bq_s", [P, HG], F32, kind="ExternalInput")
    bk_d = nc.dram_tensor("bk_s", [P, HG], F32, kind="ExternalInput")
    bv_d = nc.dram_tensor("bv_s", [P, HG], F32, kind="ExternalInput")
    bo_d = nc.dram_tensor("bo_full", [1, D], F32, kind="ExternalInput")
    if use_mask:
        mask_d = nc.dram_tensor("mask_g", [1, S], I32, kind="ExternalInput")
    # fused [mean | out] slice: one ReduceScatter + one writeback per qb
    comb_d = nc.dram_tensor("comb_slice", [4, P, S + D], BF16,
                            kind="ExternalOutput")

    groups8 = [[0, 1, 2, 3, 4, 5, 6, 7]]
    groups4 = [[0, 1, 2, 3], [4, 5, 6, 7]]
    WIDX = {"q": 0, "k": 1, "v": 2, "o": 3}

    with tile.TileContext(nc) as tc:
        with tc.tile_pool(name="dram", bufs=1, space="DRAM") as dram, \
             tc.tile_pool(name="const", bufs=1) as const:

            # internal DRAM staging
            cc_in = dram.tile([4], F32)
            cc_out = dram.tile([4], F32)
            comb_part = [dram.tile([512, S + D], BF16, name=f"comb_part{i}")
                         for i in range(4)]
            comb_rs = [dram.tile([P, S + D], BF16, name=f"comb_rs{i}")
                       for i in range(4)]

            bias_sb = {}
            for nm, d in (("q", bq_d), ("k", bk_d), ("v", bv_d)):
                t = const.tile([P, HG], F32, name=f"bias_{nm}")
                nc.sync.dma_start(out=t[:], in_=d.ap()[:])
                bias_sb[nm] = t
            # bo/4: each of the 4 cores adds a quarter so the ReduceScatter
            # sum contributes exactly bo
            bo_bc = const.tile([P, D], BF16)
            nc.gpsimd.dma_start(out=bo_bc[0:1, :], in_=bo_d.ap()[:])
            nc.vector.tensor_scalar(out=bo_bc[0:1, :], in0=bo_bc[0:1, :],
                                    scalar1=0.25, scalar2=None, op0=ALU.mult)
            nc.gpsimd.partition_broadcast(bo_bc[:], bo_bc[0:1, :])

            # ---------- Phase W: |w| slab sums -> AllReduce -> scales ----------
            acc4 = const.tile([P, 4], F32)
            ones128 = const.tile([P, 1], F32)
            nc.vector.memset(ones128[:], 1.0)
            with tc.tile_pool(name="slab", bufs=2) as slabp, \
                 tc.tile_pool(name="w0psum", bufs=1, space="PSUM") as w0p:
                for wi in range(4):
                    sl = slabp.tile([P, 2, D], F32, tag="slab")
                    nc.sync.dma_start(
                        out=sl[:],
                        in_=wslab_d.ap()[wi].rearrange("(ss p) d -> p ss d",
                                                       p=P))
                    dummy = slabp.tile([P, 2, D], F32, tag="dummy")
                    nc.scalar.activation(dummy[:], sl[:], ACTF.Abs,
                                         accum_out=acc4[:, wi:wi + 1])
                ps4 = w0p.tile([4, 1], F32, tag="ps4")
                nc.tensor.matmul(ps4[:], acc4[:], ones128[:], start=True,
                                 stop=True)
                sums_sb = const.tile([4, 1], F32)
                nc.scalar.copy(out=sums_sb[:], in_=ps4[:])
            nc.sync.dma_start(out=cc_in[:], in_=sums_sb[:])
            cc("AllReduce", ALU.add, groups8, [cc_in[:]], [cc_out[:]])
            rsum = const.tile([1, 4], F32)
            nc.sync.dma_start(out=rsum[:], in_=cc_out[:])

            scale4 = const.tile([1, 4], F32)
            nc.vector.tensor_scalar(out=scale4[:], in0=rsum[:],
                                    scalar1=float(np.float32(1.0 / (D * D))),
                                    scalar2=1e-5, op0=ALU.mult, op1=ALU.max)
            nc.vector.tensor_scalar(out=scale4[:], in0=scale4[:],
                                    scalar1=1000.0, scalar2=None, op0=ALU.min)
            thr4 = const.tile([1, 4], F32)
            nc.vector.tensor_scalar(out=thr4[:], in0=scale4[:],
                                    scalar1=float(THRESH), scalar2=None,
                                    op0=ALU.mult)
            nthr4 = const.tile([1, 4], F32)
            nc.vector.tensor_scalar(out=nthr4[:], in0=thr4[:], scalar1=-1.0,
                                    scalar2=None, op0=ALU.mult)
            scale_c4 = const.tile([1, 4], F32)
            nc.vector.tensor_scalar(out=scale_c4[:], in0=scale4[:],
                                    scalar1=float(C_SCALE), scalar2=None,
                                    op0=ALU.mult)
            # 16*scale_o for the attT evacuation (undo probs/16, fold wo scale)
            scale_o16 = const.tile([1, 1], F32)
            nc.vector.tensor_scalar(out=scale_o16[:], in0=scale4[:, 3:4],
                                    scalar1=16.0, scalar2=None, op0=ALU.mult)

            def bcast(src_ap, name):
                t = const.tile([P, 1], F32, name=name)
                nc.gpsimd.partition_broadcast(t[:], src_ap)
                return t

            thr_bc = [bcast(thr4[:, wi:wi + 1], f"thr{wi}")
                      for wi in range(4)]
            nthr_bc = [bcast(nthr4[:, wi:wi + 1], f"nthr{wi}")
                       for wi in range(4)]
            sc_bc = [bcast(scale4[:, wi:wi + 1], f"sc{wi}")
                     for wi in range(4)]
            scq_bc = bcast(scale_c4[:, 0:1], "scqc")
            sco16_bc = bcast(scale_o16[:], "sco16")
            # bias_v * scale_o (fold wo scale into attT)
            bv_sc = const.tile([P, HG], F32)
            nc.vector.tensor_scalar(out=bv_sc[:], in0=bias_sb["v"][:],
                                    scalar1=sc_bc[3][:], scalar2=None,
                                    op0=ALU.mult)

            # ---------- ternarize one pre-transposed weight chunk-by-chunk ---
            # tern = ((w >= -t) - 1) + (w > t)  in {-1, 0, 1}
            def ternarize_T(nm, wT_tile, scratch, nchunks, free):
                wi = WIDX[nm]
                for ci in range(nchunks):
                    for cj in range(free // 512):
                        cjl = slice(cj * 512, (cj + 1) * 512)
                        wnat = scratch.tile([P, 512], F32, tag="wnat")
                        nc.sync.dma_start(
                            out=wnat[:],
                            in_=w_in[nm].ap()[ci * P:(ci + 1) * P, cjl])
                        tmp = scratch.tile([P, 512], BF16, tag="terntmp")
                        nc.vector.tensor_scalar(out=tmp[:], in0=wnat[:],
                                                scalar1=nthr_bc[wi][:],
                                                scalar2=-1.0, op0=ALU.is_ge,
                                                op1=ALU.add)
                        gt = scratch.tile([P, 512], BF16, tag="terngt")
                        nc.vector.tensor_scalar(out=gt[:], in0=wnat[:],
                                                scalar1=thr_bc[wi][:],
                                                scalar2=None, op0=ALU.is_gt)
                        nc.vector.tensor_tensor(out=wT_tile[:, ci, cjl],
                                                in0=tmp[:], in1=gt[:],
                                                op=ALU.add)

            with tc.tile_pool(name="kv", bufs=1) as kvp:
                kT_hi = kvp.tile([P, HG, S], BF16)        # [d', h, s]
                kT_lo = kvp.tile([P, HG, S], BF16)
                qT_hi = kvp.tile([P, HG, S], BF16)
                qT_lo = kvp.tile([P, HG, S], BF16)
                v_sb = kvp.tile([P, 16, OS], BF16)        # [s_p, st, o]

                # ---------- Phase X: projections ----------
                with tc.tile_pool(name="wt", bufs=1) as wtp, \
                     tc.tile_pool(name="xnat", bufs=2) as xnatp, \
                     tc.tile_pool(name="xhl", bufs=1) as xhlp, \
                     tc.tile_pool(name="xt", bufs=2) as xtp, \
                     tc.tile_pool(name="qstage", bufs=4) as qstg, \
                     tc.tile_pool(name="wscratch", bufs=2) as wscr, \
                     tc.tile_pool(name="pmm", bufs=6, space="PSUM") as pmm:

                    def prep_sb(nm, x_d, sb):
                        xTh = xtp.tile([P, NDT, 512], BF16, tag="xTh")
                        xTl = None
                        if nm != "v":
                            xTl = xtp.tile([P, NDT, 512], BF16, tag="xTl")
                        for ss in range(4):
                            r0 = sb * 512 + ss * P
                            ssl = slice(ss * P, (ss + 1) * P)
                            xc = xnatp.tile([P, D], F32, tag="xc")
                            nc.sync.dma_start(
                                out=xc[:], in_=x_d.ap()[r0:r0 + P, :])
                            xh = xhlp.tile([P, D], BF16, tag="xh")
                            nc.scalar.copy(out=xh[:], in_=xc[:])
                            nc.sync.dma_start_transpose(
                                out=xTh[:, :, ssl], in_=xh[:])
                            if nm != "v":
                                xl = xhlp.tile([P, D], BF16, tag="xl")
                                nc.vector.tensor_tensor(
                                    out=xl[:], in0=xc[:], in1=xh[:],
                                    op=ALU.subtract)
                                nc.sync.dma_start_transpose(
                                    out=xTl[:, :, ssl], in_=xl[:])
                        return xTh, xTl

                    for nm, x_d in (("q", xq_d), ("k", xk_d), ("v", xv_d)):
                        wT = wtp.tile([P, NDT, OS], BF16, tag="wT",
                                      name=f"wT_{nm}")
                        # issue the first x block's loads+transposes before
                        # the weight ternarize so they overlap the scale
                        # AllReduce at kernel start
                        nxt = prep_sb(nm, x_d, 0)
                        ternarize_T(nm, wT, wscr, NDT, OS)
                        for sb in range(4):
                            xTh, xTl = nxt
                            if sb < 3:
                                nxt = prep_sb(nm, x_d, sb + 1)
                            if nm == "v":
                                for st_i in range(4):
                                    pp = pmm.tile([P, OS], F32, tag="pp")
                                    stl = slice(st_i * P, (st_i + 1) * P)
                                    for dt_i in range(NDT):
                                        nc.tensor.matmul(
                                            pp[:], xTh[:, dt_i, stl],
                                            wT[:, dt_i, :],
                                            start=(dt_i == 0),
                                            stop=(dt_i == NDT - 1))
                                    nc.scalar.activation(
                                        v_sb[:, sb * 4 + st_i, :], pp[:],
                                        ACTF.Copy, scale=sc_bc[2][:])
                            else:
                                for ot in range(HG):
                                    pp = pmm.tile([P, 512], F32, tag="pp")
                                    otl = slice(ot * P, (ot + 1) * P)
                                    for dt_i in range(NDT):
                                        nc.tensor.matmul(
                                            pp[:], wT[:, dt_i, otl],
                                            xTh[:, dt_i, :],
                                            start=(dt_i == 0), stop=False)
                                        nc.tensor.matmul(
                                            pp[:], wT[:, dt_i, otl],
                                            xTl[:, dt_i, :],
                                            start=False,
                                            stop=(dt_i == NDT - 1))
                                    ev = qstg.tile([P, 512], F32, tag="ev")
                                    if nm == "q":
                                        nc.scalar.activation(
                                            ev[:], pp[:], ACTF.Identity,
                                            bias=bias_sb["q"][:, ot:ot + 1],
                                            scale=scq_bc[:])
                                    else:
                                        nc.scalar.activation(
                                            ev[:], pp[:], ACTF.Identity,
                                            bias=bias_sb["k"][:, ot:ot + 1],
                                            scale=sc_bc[1][:])
                                    th, tl = ((qT_hi, qT_lo)
                                              if nm == "q"
                                              else (kT_hi, kT_lo))
                                    sl5 = slice(sb * 512, (sb + 1) * 512)
                                    nc.scalar.copy(
                                        out=th[:, ot, sl5], in_=ev[:])
                                    nc.vector.tensor_tensor(
                                        out=tl[:, ot, sl5],
                                        in0=ev[:], in1=th[:, ot, sl5],
                                        op=ALU.subtract)

                # ---------- Phase A: attention + per-qb output projection ----
                if phases >= 2:
                    with tc.tile_pool(name="wop", bufs=1) as wop, \
                         tc.tile_pool(name="wos", bufs=2) as wos, \
                         tc.tile_pool(name="accp", bufs=1) as accp, \
                         tc.tile_pool(name="ptld", bufs=2) as ptld, \
                         tc.tile_pool(name="probs", bufs=3) as probsp, \
                         tc.tile_pool(name="attts", bufs=2) as atttp, \
                         tc.tile_pool(name="smax", bufs=4) as smaxp, \
                         tc.tile_pool(name="outs", bufs=2) as outsp, \
                         tc.tile_pool(name="scp", bufs=6, space="PSUM") as scp, \
                         tc.tile_pool(name="avp", bufs=1, space="PSUM") as avp, \
                         tc.tile_pool(name="pop", bufs=1, space="PSUM") as pop:

                        woTb = wop.tile([P, HG, D], BF16)  # [d'_p, dt, o]
                        ternarize_T("o", woTb, wos, HG, D)
                        if use_mask:
                            mbias = accp.tile([P, S], F32, tag="mbias")
                            nc.gpsimd.dma_start(out=mbias[0:1, :],
                                                in_=mask_d.ap()[:])
                            nc.vector.tensor_scalar(
                                out=mbias[0:1, :], in0=mbias[0:1, :],
                                scalar1=-1.0, scalar2=1e9,
                                op0=ALU.add, op1=ALU.mult)
                            nc.gpsimd.partition_broadcast(mbias[:],
                                                          mbias[0:1, :])
                        for qb in range(4):
                            acc = accp.tile([P, 4, S], F32, tag="acc")
                            attT_sb = atttp.tile([P, HG, 512], BF16,
                                                 tag="attT")
                            for h in range(HG):
                                probsT = ptld.tile([P, 16, 512], BF16,
                                                   tag="pT")
                                for qt in range(4):
                                    psc = [scp.tile([P, 512], F32, tag="sc",
                                                    name=f"sc{kb}")
                                           for kb in range(4)]
                                    q0c = qb * 512
                                    qcol = slice(q0c + qt * P,
                                                 q0c + (qt + 1) * P)
                                    for vi, (lh, kt_sb) in enumerate(
                                            ((qT_hi, kT_hi), (qT_hi, kT_lo),
                                             (qT_lo, kT_hi))):
                                        for kb in range(4):
                                            kbs = slice(kb * 512,
                                                        (kb + 1) * 512)
                                            nc.tensor.matmul(
                                                psc[kb][:],
                                                lh[:, h, qcol],
                                                kt_sb[:, h, kbs],
                                                start=(vi == 0),
                                                stop=(vi == 2))
                                    nm4 = smaxp.tile([P, 4], F32, tag="nm4")
                                    for kb in range(4):
                                        if use_mask:
                                            nc.vector.tensor_tensor(
                                                out=psc[kb][:],
                                                in0=psc[kb][:],
                                                in1=mbias[:,
                                                          kb * 512:
                                                          (kb + 1) * 512],
                                                op=ALU.add)
                                        nc.vector.tensor_reduce(
                                            out=nm4[:, kb:kb + 1],
                                            in_=psc[kb][:],
                                            axis=AX.X, op=ALU.max)
                                    nmax = smaxp.tile([P, 1], F32,
                                                      tag="nmax")
                                    nc.vector.tensor_reduce(
                                        out=nmax[:], in_=nm4[:], axis=AX.X,
                                        op=ALU.max, negate=True)
                                    probs = probsp.tile([P, S], BF16,
                                                        tag="probs")
                                    den4 = smaxp.tile([P, 4], F32,
                                                      tag="den4")
                                    for kb in range(4):
                                        nc.scalar.activation(
                                            probs[:, kb * 512:
                                                  (kb + 1) * 512],
                                            psc[kb][:], ACTF.Exp,
                                            bias=nmax[:], scale=1.0,
                                            accum_out=den4[:, kb:kb + 1])
                                    den = smaxp.tile([P, 1], F32, tag="den")
                                    nc.vector.tensor_reduce(
                                        out=den[:], in_=den4[:], axis=AX.X,
                                        op=ALU.add)
                                    nc.vector.tensor_scalar(
                                        out=den[:], in0=den[:],
                                        scalar1=16.0, scalar2=None,
                                        op0=ALU.mult)
                                    r16 = smaxp.tile([P, 1], F32, tag="r16")
                                    nc.vector.reciprocal(out=r16[:],
                                                         in_=den[:])
                                    nc.vector.tensor_scalar(
                                        out=probs[:], in0=probs[:],
                                        scalar1=r16[:], scalar2=None,
                                        op0=ALU.mult)
                                    if h == 0:
                                        nc.vector.tensor_copy(
                                            out=acc[:, qt, :], in_=probs[:])
                                    else:
                                        nc.gpsimd.tensor_tensor(
                                            out=acc[:, qt, :],
                                            in0=acc[:, qt, :],
                                            in1=probs[:], op=ALU.add)
                                    nc.sync.dma_start_transpose(
                                        out=probsT[:, :,
                                                   qt * P:(qt + 1) * P],
                                        in_=probs[:])
                                pav = avp.tile([P, 512], F32, tag="av")
                                for kt in range(16):
                                    nc.tensor.matmul(
                                        pav[:],
                                        v_sb[:, kt, h * P:(h + 1) * P],
                                        probsT[:, kt, :],
                                        start=(kt == 0), stop=(kt == 15))
                                nc.vector.tensor_scalar(
                                    out=attT_sb[:, h, :], in0=pav[:],
                                    scalar1=sco16_bc[:],
                                    scalar2=bv_sc[:, h:h + 1],
                                    op0=ALU.mult, op1=ALU.add)
                            # bf16 partial mean into the fused buffer,
                            # issued before the output projection
                            nc.gpsimd.dma_start(
                                out=comb_part[qb][:, 0:S]
                                .rearrange("(qt p) k -> p qt k", p=P),
                                in_=acc[:])
                            # ---- output projection partial for this qb ----
                            for qt in range(4):
                                osb = outsp.tile([P, D], BF16, tag="osb")
                                qtl = slice(qt * P, (qt + 1) * P)
                                for oc in range(4):
                                    ocl = slice(oc * 512, (oc + 1) * 512)
                                    po = pop.tile([P, 512], F32, tag="po")
                                    for dt in range(HG):
                                        nc.tensor.matmul(
                                            po[:], attT_sb[:, dt, qtl],
                                            woTb[:, dt, ocl],
                                            start=(dt == 0),
                                            stop=(dt == HG - 1))
                                    nc.vector.tensor_tensor(
                                        out=osb[:, ocl], in0=po[:],
                                        in1=bo_bc[:, ocl], op=ALU.add)
                                nc.sync.dma_start(
                                    out=comb_part[qb][qt * P:(qt + 1) * P,
                                                      S:S + D],
                                    in_=osb[:])
                            cc("ReduceScatter", ALU.add, groups4,
                               [comb_part[qb][:]], [comb_rs[qb][:]])
                            nc.sync.dma_start(out=comb_d.ap()[qb],
                                              in_=comb_rs[qb][:])

    nc.compile()
    return nc


def kernel(**inputs):
    query = np.ascontiguousarray(inputs["query"], dtype=np.float32)
    key = np.ascontiguousarray(inputs["key"], dtype=np.float32)
    value = np.ascontiguousarray(inputs["value"], dtype=np.float32)
    mask = np.asarray(inputs["mask"])
    ws = {n: np.ascontiguousarray(inputs[n], dtype=np.float32)
          for n in ("wq", "wk", "wv", "wo")}
    bs = {n: np.ascontiguousarray(inputs[n], dtype=np.float32)
          for n in ("bq", "bk", "bv", "bo")}

    use_mask = not bool(np.all(mask == 1))
    if use_mask not in _CACHE:
        _CACHE[use_mask] = _build(use_mask)
    nc = _CACHE[use_mask]

    in_maps = []
    for cid in range(NCORES):
        g, j = divmod(cid, 4)
        sl = slice(OS * j, OS * (j + 1))
        m = {
            "xq": query[g], "xk": key[g], "xv": value[g],
            "w_slab": np.stack([ws[n][256 * cid:256 * (cid + 1), :]
                                for n in ("wq", "wk", "wv", "wo")]),
            "wqT_s": np.ascontiguousarray(ws["wq"][sl].T),
            "wkT_s": np.ascontiguousarray(ws["wk"][sl].T),
            "wvT_s": np.ascontiguousarray(ws["wv"][sl].T),
            "woT_s": np.ascontiguousarray(ws["wo"][:, sl].T),
            "bq_s": np.ascontiguousarray(bs["bq"][sl].reshape(HG, P).T),
            "bk_s": np.ascontiguousarray(bs["bk"][sl].reshape(HG, P).T),
            "bv_s": np.ascontiguousarray(bs["bv"][sl].reshape(HG, P).T),
            "bo_full": bs["bo"].reshape(1, D),
        }
        if use_mask:
            m["mask_g"] = np.ascontiguousarray(
                mask[g], dtype=np.int32).reshape(1, S)
        in_maps.append(m)

    global _last_in_maps
    _last_in_maps = in_maps

    from concourse.bass_utils import run_bass_kernel_spmd
    res = run_bass_kernel_spmd(nc, in_maps, core_ids=list(range(NCORES)))

    out = np.empty((B, S, D), np.float32)
    attn_mean = np.empty((B, S, S), np.float32)
    for cid in range(NCORES):
        g, j = divmod(cid, 4)
        cs = np.asarray(res.results[cid]["comb_slice"]).astype(np.float32)
        for qb in range(4):
            r0 = qb * 512 + P * j
            attn_mean[g][r0:r0 + P, :] = cs[qb][:, 0:S]
            out[g][r0:r0 + P, :] = cs[qb][:, S:S + D]
    return out, attn_mean



# revision 4
# speedup vs baseline: 1.4956x; 1.4956x over previous
"""BitNet attention TRN2 kernel v2: 8-core SPMD (2 batch groups x 4 head groups).

Per core cid = 4*g + j (g = batch index, j = head-group index):
  - host prep: weights are ternarized bit-exactly with the reference's jax
    fp32 formula, scaled, sliced, transposed, and sent as fp16; x inputs are
    hi/lo-split fp16, transposed, and blocked on host (no on-chip transposes,
    no on-chip ternarize, no scale AllReduce).
  - projections: fp16 matmuls, q/k in x-hi + x-lo passes for fp32-grade
    precision, v single-pass; outputs written directly as fp16 hi/lo.
  - attention: per-kb (512-key block) max -> exp pipeline so PSUM banks free
    early; softmax normalization folded into per-partition scalars applied to
    fp16 probs; probs transposed via DMA xbar; attn-mean accumulated fp16
    (host divides by 16); output projection partial summed over the core's
    512 attended dims.
  - per-qb fused [mean | out] fp16 slice ReduceScattered over the 4-core
    batch group.
"""

import os

import numpy as np

os.environ.setdefault("NEURON_RT_RESET_CORES", "1")

B, S, D, H = 2, 2048, 2048, 16
HD = D // H            # 128 head dim
HG = H // 4            # 4 heads per core
OS = HG * HD           # 512-wide slice per core
P = 128
NCORES = 8
NDT = D // P           # 16 contraction tiles
C_SCALE = np.float32(1.0 / np.sqrt(HD))

_CACHE = {}
_last_in_maps = None


def _build(use_mask: bool, zero_bias: bool, pp: int = 2, sp: int = 3,
           single: bool = False):
    import concourse.mybir as mybir
    import concourse.tile as tile
    from concourse import bacc

    F32 = mybir.dt.float32
    F16 = mybir.dt.float16
    I32 = mybir.dt.int32
    AX = mybir.AxisListType
    ALU = mybir.AluOpType
    ACTF = mybir.ActivationFunctionType

    nc = bacc.Bacc("TRN2", target_bir_lowering=False, debug=False,
                   num_devices=1 if single else NCORES)

    def cc(kind, op, groups, ins, outs):
        if not single:
            nc.gpsimd.collective_compute(kind, op, replica_groups=groups,
                                         ins=ins, outs=outs)
            return
        src_ap, dst_ap = ins[0], outs[0]
        if kind == "ReduceScatter":
            nc.gpsimd.dma_start(out=dst_ap, in_=src_ap[0:dst_ap.shape[0]])
        else:
            nc.gpsimd.dma_start(out=dst_ap, in_=src_ap)

    # ---- I/O (all per-core slices prepped on host) ----
    x_in = {}
    for nm in ("q", "k", "v"):
        x_in[nm, "h"] = nc.dram_tensor(f"x{nm}h", [4, P, NDT, 512], F16,
                                       kind="ExternalInput")
        if pp == 2 and nm != "v":
            x_in[nm, "l"] = nc.dram_tensor(f"x{nm}l", [4, P, NDT, 512], F16,
                                           kind="ExternalInput")
    w_in = {
        "q": nc.dram_tensor("tq", [P, NDT, OS], F16, kind="ExternalInput"),
        "k": nc.dram_tensor("tk", [P, NDT, OS], F16, kind="ExternalInput"),
        "v": nc.dram_tensor("tv", [P, NDT, OS], F16, kind="ExternalInput"),
        "o": nc.dram_tensor("to", [P, HG, D], F16, kind="ExternalInput"),
    }
    if not zero_bias:
        bq_d = nc.dram_tensor("bq_s", [P, HG], F32, kind="ExternalInput")
        bk_d = nc.dram_tensor("bk_s", [P, HG], F32, kind="ExternalInput")
        bv_d = nc.dram_tensor("bvsc", [P, HG], F32, kind="ExternalInput")
        bo_d = nc.dram_tensor("bo_full", [1, D], F32, kind="ExternalInput")
    if use_mask:
        mask_d = nc.dram_tensor("mask_g", [1, S], I32, kind="ExternalInput")
    # fused [mean | out] slice: one ReduceScatter + one writeback per qb
    comb_d = nc.dram_tensor("comb_slice", [4, P, S + D], F16,
                            kind="ExternalOutput")

    groups4 = [[0, 1, 2, 3], [4, 5, 6, 7]]

    with tile.TileContext(nc) as tc:
        with tc.tile_pool(name="dram", bufs=1, space="DRAM") as dram, \
             tc.tile_pool(name="const", bufs=1) as const:

            comb_part = [dram.tile([512, S + D], F16, name=f"comb_part{i}")
                         for i in range(4)]
            comb_rs = [dram.tile([P, S + D], F16, name=f"comb_rs{i}")
                       for i in range(4)]

            if not zero_bias:
                bias_sb = {}
                for nm, d in (("q", bq_d), ("k", bk_d), ("v", bv_d)):
                    t = const.tile([P, HG], F32, name=f"bias_{nm}")
                    nc.sync.dma_start(out=t[:], in_=d.ap()[:])
                    bias_sb[nm] = t
                bo_bc = const.tile([P, D], F32)
                nc.gpsimd.dma_start(out=bo_bc[0:1, :], in_=bo_d.ap()[:])
                nc.gpsimd.partition_broadcast(bo_bc[:], bo_bc[0:1, :])

            # ---------- persistent projection outputs ----------
            with tc.tile_pool(name="kv", bufs=1) as kvp:
                kT_hi = kvp.tile([P, HG, S], F16)        # [d', h, s]
                kT_lo = kvp.tile([P, HG, S], F16)
                qT_hi = kvp.tile([P, HG, S], F16)
                qT_lo = kvp.tile([P, HG, S], F16)
                v_sb = kvp.tile([P, 16, OS], F16)        # [s_p, st, o]

                # ---------- Phase X: projections ----------
                with tc.tile_pool(name="wt", bufs=2) as wtp, \
                     tc.tile_pool(name="xt", bufs=2) as xtp, \
                     tc.tile_pool(name="qstage", bufs=3) as qstg, \
                     tc.tile_pool(name="pmm", bufs=4, space="PSUM") as pmm:

                    for nm in ("q", "k", "v"):
                        wT = wtp.tile([P, NDT, OS], F16, tag="wT",
                                      name=f"wT_{nm}")
                        nc.sync.dma_start(out=wT[:], in_=w_in[nm].ap()[:])
                        for sb in range(4):
                            xTh = xtp.tile([P, NDT, 512], F16, tag="xTh")
                            nc.sync.dma_start(
                                out=xTh[:], in_=x_in[nm, "h"].ap()[sb])
                            xTl = None
                            if pp == 2 and nm != "v":
                                xTl = xtp.tile([P, NDT, 512], F16, tag="xTl")
                                nc.sync.dma_start(
                                    out=xTl[:], in_=x_in[nm, "l"].ap()[sb])
                            if nm == "v":
                                for st_i in range(4):
                                    pv = pmm.tile([P, OS], F32, tag="pp")
                                    stl = slice(st_i * P, (st_i + 1) * P)
                                    for dt_i in range(NDT):
                                        nc.tensor.matmul(
                                            pv[:], xTh[:, dt_i, stl],
                                            wT[:, dt_i, :],
                                            start=(dt_i == 0),
                                            stop=(dt_i == NDT - 1))
                                    nc.scalar.activation(
                                        v_sb[:, sb * 4 + st_i, :], pv[:],
                                        ACTF.Copy)
                            else:
                                th, tl = ((qT_hi, qT_lo) if nm == "q"
                                          else (kT_hi, kT_lo))
                                for ot in range(HG):
                                    pq = pmm.tile([P, 512], F32, tag="pp")
                                    otl = slice(ot * P, (ot + 1) * P)
                                    for dt_i in range(NDT):
                                        nc.tensor.matmul(
                                            pq[:], wT[:, dt_i, otl],
                                            xTh[:, dt_i, :],
                                            start=(dt_i == 0),
                                            stop=(pp == 1
                                                  and dt_i == NDT - 1))
                                        if pp == 2:
                                            nc.tensor.matmul(
                                                pq[:], wT[:, dt_i, otl],
                                                xTl[:, dt_i, :],
                                                start=False,
                                                stop=(dt_i == NDT - 1))
                                    sl5 = slice(sb * 512, (sb + 1) * 512)
                                    if zero_bias:
                                        nc.scalar.activation(
                                            th[:, ot, sl5], pq[:], ACTF.Copy)
                                        nc.vector.tensor_tensor(
                                            out=tl[:, ot, sl5], in0=pq[:],
                                            in1=th[:, ot, sl5],
                                            op=ALU.subtract)
                                    else:
                                        bk = bias_sb[nm][:, ot:ot + 1]
                                        ev = qstg.tile([P, 512], F32,
                                                       tag="ev")
                                        nc.scalar.activation(
                                            ev[:], pq[:], ACTF.Identity,
                                            bias=bk)
                                        nc.scalar.activation(
                                            th[:, ot, sl5], ev[:], ACTF.Copy)
                                        nc.vector.tensor_tensor(
                                            out=tl[:, ot, sl5], in0=ev[:],
                                            in1=th[:, ot, sl5],
                                            op=ALU.subtract)

                # ---------- Phase A: attention + output projection ----------
                with tc.tile_pool(name="wop", bufs=1) as wop, \
                     tc.tile_pool(name="accp", bufs=1) as accp, \
                     tc.tile_pool(name="ptld", bufs=2) as ptld, \
                     tc.tile_pool(name="probs", bufs=3) as probsp, \
                     tc.tile_pool(name="attts", bufs=2) as atttp, \
                     tc.tile_pool(name="smax", bufs=4) as smaxp, \
                     tc.tile_pool(name="outs", bufs=3) as outsp, \
                     tc.tile_pool(name="scp", bufs=4, space="PSUM") as scp, \
                     tc.tile_pool(name="avp", bufs=2, space="PSUM") as avp, \
                     tc.tile_pool(name="pop", bufs=1, space="PSUM") as pop:

                    woTb = wop.tile([P, HG, D], F16)  # [d'_p, dt, o]
                    nc.sync.dma_start(out=woTb[:], in_=w_in["o"].ap()[:])
                    if use_mask:
                        mbias = accp.tile([P, S], F32, tag="mbias")
                        nc.gpsimd.dma_start(out=mbias[0:1, :],
                                            in_=mask_d.ap()[:])
                        nc.vector.tensor_scalar(
                            out=mbias[0:1, :], in0=mbias[0:1, :],
                            scalar1=-1.0, scalar2=1e9,
                            op0=ALU.add, op1=ALU.mult)
                        nc.gpsimd.partition_broadcast(mbias[:],
                                                      mbias[0:1, :])
                    for qb in range(4):
                        acc = accp.tile([P, 4, S], F16, tag="acc")
                        attT_sb = atttp.tile([P, HG, 512], F16, tag="attT")
                        q0c = qb * 512
                        for h in range(HG):
                            probsT = ptld.tile([P, 16, 512], F16, tag="pT")
                            for qt in range(4):
                                qcol = slice(q0c + qt * P, q0c + (qt + 1) * P)
                                probs = probsp.tile([P, S], F16, tag="probs")
                                nm4 = smaxp.tile([P, 4], F32, tag="nm4")
                                den4 = smaxp.tile([P, 4], F32, tag="den4")
                                for kb in range(4):
                                    kbs = slice(kb * 512, (kb + 1) * 512)
                                    psc = scp.tile([P, 512], F32, tag="sc")
                                    nc.tensor.matmul(
                                        psc[:], qT_hi[:, h, qcol],
                                        kT_hi[:, h, kbs],
                                        start=True, stop=(sp == 1))
                                    if sp >= 2:
                                        nc.tensor.matmul(
                                            psc[:], qT_lo[:, h, qcol],
                                            kT_hi[:, h, kbs],
                                            start=False, stop=(sp == 2))
                                    if sp >= 3:
                                        nc.tensor.matmul(
                                            psc[:], qT_hi[:, h, qcol],
                                            kT_lo[:, h, kbs],
                                            start=False, stop=True)
                                    if use_mask:
                                        nc.vector.tensor_tensor(
                                            out=psc[:], in0=psc[:],
                                            in1=mbias[:, kbs], op=ALU.add)
                                    nc.vector.tensor_reduce(
                                        out=nm4[:, kb:kb + 1], in_=psc[:],
                                        axis=AX.X, op=ALU.max, negate=True)
                                    nc.scalar.activation(
                                        probs[:, kbs], psc[:], ACTF.Exp,
                                        bias=nm4[:, kb:kb + 1], scale=1.0,
                                        accum_out=den4[:, kb:kb + 1])
                                # combine per-kb stats:
                                # M = max_kb m_kb; f_kb = exp(m_kb - M)
                                # den = sum_kb den_kb * f_kb ; sc_kb = f_kb/den
                                mneg = smaxp.tile([P, 1], F32, tag="mneg")
                                nc.vector.tensor_reduce(
                                    out=mneg[:], in_=nm4[:], axis=AX.X,
                                    op=ALU.min)
                                f4 = smaxp.tile([P, 4], F32, tag="f4")
                                nc.scalar.activation(
                                    f4[:], nm4[:], ACTF.Exp,
                                    bias=mneg[:], scale=-1.0)
                                tmp4 = smaxp.tile([P, 4], F32, tag="tmp4")
                                dent = smaxp.tile([P, 1], F32, tag="dent")
                                nc.vector.tensor_tensor(
                                    out=tmp4[:], in0=den4[:], in1=f4[:],
                                    op=ALU.mult)
                                nc.vector.tensor_reduce(
                                    out=dent[:], in_=tmp4[:], axis=AX.X,
                                    op=ALU.add)
                                rden = smaxp.tile([P, 1], F32, tag="rden")
                                nc.vector.reciprocal(out=rden[:],
                                                     in_=dent[:])
                                sc4 = smaxp.tile([P, 4], F32, tag="sc4")
                                nc.vector.tensor_scalar(
                                    out=sc4[:], in0=f4[:], scalar1=rden[:],
                                    scalar2=None, op0=ALU.mult)
                                for kb in range(4):
                                    kbs = slice(kb * 512, (kb + 1) * 512)
                                    nc.vector.tensor_scalar(
                                        out=probs[:, kbs], in0=probs[:, kbs],
                                        scalar1=sc4[:, kb:kb + 1],
                                        scalar2=None, op0=ALU.mult)
                                if h == 0:
                                    nc.vector.tensor_copy(
                                        out=acc[:, qt, :], in_=probs[:])
                                elif h == 1:
                                    nc.vector.tensor_tensor(
                                        out=acc[:, qt, :], in0=acc[:, qt, :],
                                        in1=probs[:], op=ALU.add)
                                else:
                                    nc.gpsimd.tensor_tensor(
                                        out=acc[:, qt, :], in0=acc[:, qt, :],
                                        in1=probs[:], op=ALU.add)
                                nc.sync.dma_start_transpose(
                                    out=probsT[:, :, qt * P:(qt + 1) * P],
                                    in_=probs[:])
                            pav = avp.tile([P, 512], F32, tag="av")
                            for kt in range(16):
                                nc.tensor.matmul(
                                    pav[:], v_sb[:, kt, h * P:(h + 1) * P],
                                    probsT[:, kt, :],
                                    start=(kt == 0), stop=(kt == 15))
                            if zero_bias:
                                nc.scalar.activation(
                                    attT_sb[:, h, :], pav[:], ACTF.Copy)
                            else:
                                nc.scalar.activation(
                                    attT_sb[:, h, :], pav[:], ACTF.Identity,
                                    bias=bias_sb["v"][:, h:h + 1])
                        # mean partial (sum of 4 heads' probs; host / 16)
                        nc.sync.dma_start(
                            out=comb_part[qb][:, 0:S]
                            .rearrange("(qt p) k -> p qt k", p=P),
                            in_=acc[:])
                        # ---- output projection partial for this qb ----
                        for qt in range(4):
                            osb = outsp.tile([P, D], F16, tag="osb")
                            qtl = slice(qt * P, (qt + 1) * P)
                            for ocp in range(2):
                                po = pop.tile([P, 2, 512], F32, tag="po")
                                for dt in range(HG):
                                    for oc2 in range(2):
                                        oc = ocp * 2 + oc2
                                        ocl = slice(oc * 512,
                                                    (oc + 1) * 512)
                                        nc.tensor.matmul(
                                            po[:, oc2, :],
                                            attT_sb[:, dt, qtl],
                                            woTb[:, dt, ocl],
                                            start=(dt == 0),
                                            stop=(dt == HG - 1))
                                opl = slice(ocp * 1024, (ocp + 1) * 1024)
                                if zero_bias:
                                    nc.scalar.activation(
                                        osb[:, opl], po[:], ACTF.Copy)
                                else:
                                    nc.vector.tensor_tensor(
                                        out=osb[:, opl], in0=po[:],
                                        in1=bo_bc[:, opl], op=ALU.add)
                            nc.sync.dma_start(
                                out=comb_part[qb][qt * P:(qt + 1) * P,
                                                  S:S + D],
                                in_=osb[:])
                        cc("ReduceScatter", ALU.add, groups4,
                           [comb_part[qb][:]], [comb_rs[qb][:]])
                        nc.sync.dma_start(out=comb_d.ap()[qb],
                                          in_=comb_rs[qb][:])

    nc.compile()
    return nc


def _ternary_like_reference(w):
    """Bit-exact replica of reference.bitnet_weight_ste's quantization
    (jax fp32 on CPU), returning the ternary {-1,0,1} matrix and scale."""
    import jax
    import jax.numpy as jnp
    cpu = jax.devices("cpu")[0]
    with jax.default_device(cpu):
        wj = jnp.asarray(w, jnp.float32)
        scale = jnp.clip(jnp.mean(jnp.abs(wj)), 1e-5, 1000.0)
        wn = jnp.clip(wj / scale, -10.0, 10.0)
        thr = np.float32(2.0 / 3.0)
        wq = jnp.where(wn > thr, 1.0, jnp.where(wn < -thr, -1.0, 0.0))
        return np.asarray(wq, np.float32), np.float32(scale)


def _xT_blocks(x, dtype=np.float16):
    """[S, D] fp32 -> hi/lo fp16 [4, P, NDT, 512] transposed + blocked."""
    xh = x.astype(dtype)
    xl = (x - xh.astype(np.float32)).astype(dtype)

    def blk(a):
        # arr[sb, dp, dt, sc] = a[s, d], d = dt*128+dp, s = sb*512+sc
        return np.ascontiguousarray(
            a.T.reshape(NDT, P, 4, 512).transpose(2, 1, 0, 3))
    return blk(xh), blk(xl)


def _wT_blocks(t_scaled):
    """[OS rows, D] scaled ternary -> [P, NDT, OS] fp16 (transposed)."""
    return np.ascontiguousarray(
        t_scaled.T.reshape(NDT, P, OS).transpose(1, 0, 2)
        .astype(np.float16))


def kernel(**inputs):
    global _last_in_maps
    query = np.ascontiguousarray(inputs["query"], dtype=np.float32)
    key = np.ascontiguousarray(inputs["key"], dtype=np.float32)
    value = np.ascontiguousarray(inputs["value"], dtype=np.float32)
    mask = np.asarray(inputs["mask"])
    bs = {n: np.ascontiguousarray(inputs[n], dtype=np.float32)
          for n in ("bq", "bk", "bv", "bo")}

    tern = {}
    for n in ("wq", "wk", "wv", "wo"):
        tern[n] = _ternary_like_reference(
            np.ascontiguousarray(inputs[n], dtype=np.float32))

    use_mask = not bool(np.all(mask == 1))
    zero_bias = all(bool(np.all(bs[n] == 0)) for n in bs)
    cfg = (use_mask, zero_bias)
    if cfg not in _CACHE:
        _CACHE[cfg] = _build(use_mask, zero_bias)
    nc = _CACHE[cfg]

    # host prep shared across the 4 cores of each batch group
    xq = [None, None]
    xk = [None, None]
    xv = [None, None]
    for g in range(B):
        xq[g] = _xT_blocks(query[g])
        xk[g] = _xT_blocks(key[g])
        xv[g] = _xT_blocks(value[g])

    sq = np.float32(tern["wq"][1] * C_SCALE)
    in_maps = []
    for cid in range(NCORES):
        g, j = divmod(cid, 4)
        sl = slice(OS * j, OS * (j + 1))
        m = {
            "xqh": xq[g][0], "xql": xq[g][1],
            "xkh": xk[g][0], "xkl": xk[g][1],
            "xvh": xv[g][0],
            "tq": _wT_blocks(tern["wq"][0][sl, :] * sq),
            "tk": _wT_blocks(tern["wk"][0][sl, :] * tern["wk"][1]),
            "tv": _wT_blocks(tern["wv"][0][sl, :] * tern["wv"][1]),
            "to": np.ascontiguousarray(
                (tern["wo"][0][:, sl] * tern["wo"][1]).T
                .reshape(HG, P, D).transpose(1, 0, 2).astype(np.float16)),
        }
        if not zero_bias:
            m["bq_s"] = np.ascontiguousarray(
                bs["bq"][sl].reshape(HG, P).T) * C_SCALE  # match q scaling
            m["bk_s"] = np.ascontiguousarray(bs["bk"][sl].reshape(HG, P).T)
            m["bvsc"] = np.ascontiguousarray(bs["bv"][sl].reshape(HG, P).T)
            m["bo_full"] = bs["bo"].reshape(1, D)
        if use_mask:
            m["mask_g"] = np.ascontiguousarray(
                mask[g], dtype=np.int32).reshape(1, S)
        in_maps.append(m)

    _last_in_maps = in_maps

    from concourse.bass_utils import run_bass_kernel_spmd
    res = run_bass_kernel_spmd(nc, in_maps, core_ids=list(range(NCORES)))

    out = np.empty((B, S, D), np.float32)
    attn_mean = np.empty((B, S, S), np.float32)
    for cid in range(NCORES):
        g, j = divmod(cid, 4)
        cs = np.asarray(res.results[cid]["comb_slice"]).astype(np.float32)
        for qb in range(4):
            r0 = qb * 512 + P * j
            attn_mean[g][r0:r0 + P, :] = cs[qb][:, 0:S] * np.float32(1 / 16)
            out[g][r0:r0 + P, :] = cs[qb][:, S:S + D]
    return out, attn_mean
